# revision 20
# baseline (speedup 1.0000x reference)
"""Trainium2 Bass kernel for nn_Lookahead (causal-lookahead depthwise conv).

y[t, b, f] = sum_{k=0..20} x[t+k, b, f] * weight[f, k]   (zero tail padding)

Strategy:
  - Shard F=1024 across 8 cores (128 features each). Within a core, process
    feature chunks (asymmetric sizes: small head/tail chunks for pipeline
    fill/drain), fully pipelined load / compute / store via Tile pools.
  - Host pre-lays-out x per chunk as [i, s, b, f] so DMA loads land directly
    in a time-on-partitions SBUF layout [s=128, (i, b, f)]; the time conv
    becomes a banded-Toeplitz matmul on the TensorEngine:
        out[tau, (i,b)] = sum_s band_f[s, tau] * x[128*i + s, b, f]
    with band_f[s, tau] = w[f, s-tau] for 0 <= s-tau <= 20.
    Rows 0..127 of the band (L1) consume x tile i; rows 128..147 (L2)
    consume the first 20 rows of x tile i+1 via PSUM accumulation (only
    tau in [108,128) receives L2 terms; stationary is a 20x64 corner at
    PE quadrant offset 64).
  - PSUM evacuation alternates VectorE / ScalarE; y staged in half-chunk
    tiles and DMA'd out in a layout the host transposes back.
"""

import sys

sys.path.insert(0, "/opt/trn_rl_repo")

import numpy as np

T, B, F, K = 2048, 16, 1024, 21
CTX = K - 1
NCORES = 8
FC = F // NCORES  # 128 features per core
S = 128           # time-tile size (partition dim)
NI = T // S       # 16 time tiles
SB = 148          # band rows: 128 (L1) + 20 (L2)
CHUNKS = (16, 16, 32, 32, 24, 8)   # feature chunk sizes (sum = FC)
YS = 2                             # y stores per chunk
L2W = 64                           # L2 stationary cols (tau in [64,128))

assert sum(CHUNKS) == FC

_MODULE_CACHE = {}


def _offsets():
    """Per-chunk element offsets into the flat x / bands / y dram tensors."""
    xo, bo, yo = [], [], []
    x_acc = b_acc = y_acc = 0
    for fc in CHUNKS:
        xo.append(x_acc); x_acc += NI * S * B * fc
        bo.append(b_acc); b_acc += fc * SB * S
        yo.append(y_acc); y_acc += S * NI * B * fc
    return xo, bo, yo, x_acc, b_acc, y_acc


def build_module(repeat=1, bufs=(3, 2, 3, 6)):
    key = ("nc", repeat, bufs)
    if key in _MODULE_CACHE:
        return _MODULE_CACHE[key]
    import concourse.bacc as bacc
    import concourse.mybir as mybir
    from concourse.tile import TileContext

    xb, bb_, yb, pb = bufs
    dt = mybir.dt.float32
    nc = bacc.Bacc("TRN2", target_bir_lowering=False, debug=False,
                   num_devices=NCORES)

    xo, bo, yo, xn, bn, yn = _offsets()
    x_d = nc.dram_tensor("x", [xn], dt, kind="ExternalInput")
    b_d = nc.dram_tensor("bands", [bn], dt, kind="ExternalInput")
    y_d = nc.dram_tensor("y", [yn], dt, kind="ExternalOutput")

    nib = NI * B
    with TileContext(nc) as tc:
        with tc.tile_pool(name="xp", bufs=xb) as xp, \
             tc.tile_pool(name="bp", bufs=bb_) as bp, \
             tc.tile_pool(name="yp", bufs=yb) as yp, \
             tc.tile_pool(name="pp", bufs=pb, space="PSUM") as pp:
            for _ in range(repeat):
                for ci, fq in enumerate(CHUNKS):
                    PW = S + L2W
                    fq2 = fq // YS
                    xq = xp.tile([S, nib * fq], dt, tag="x")
                    bb = bp.tile([S, fq * PW], dt, tag="bb")

                    x_src = x_d.ap()[xo[ci]:xo[ci] + NI * S * B * fq] \
                        .rearrange("(i s m) -> i s m", i=NI, s=S, m=B * fq) \
                        .rearrange("i s m -> s i m")
                    nc.sync.dma_start(out=xq[:], in_=x_src)

                    b_src = b_d.ap()[bo[ci]:bo[ci] + fq * SB * S] \
                        .rearrange("(f s t) -> f s t", f=fq, s=SB, t=S)
                    bbr = bb[:].rearrange("s (f j) -> s f j", f=fq, j=PW)
                    nc.sync.dma_start(
                        out=bbr[:, :, 0:S],
                        in_=b_src[:, 0:S, :].rearrange("f s t -> s f t"))
                    nc.sync.dma_start(
                        out=bbr[0:CTX, :, S:PW],
                        in_=b_src[:, S:SB, S - L2W:S].rearrange(
                            "f s t -> s f t"))

                    xqr = xq[:].rearrange("s (i b f) -> s i b f",
                                          i=NI, b=B, f=fq)
                    ysb = None
                    ysr = None
                    for fi in range(fq):
                        if fi % fq2 == 0:
                            ysb = yp.tile([S, nib * fq2], dt, tag="y")
                            ysr = ysb[:].rearrange(
                                "t (i b f) -> t i b f", i=NI, b=B, f=fq2)
                        pt = pp.tile([S, nib], dt, tag="ps")
                        # L1: all 16 time tiles, 128-row contraction.
                        nc.tensor.matmul(
                            pt[:, :],
                            lhsT=bbr[:, fi, 0:S],
                            rhs=xqr[:, :, :, fi],
                            start=True, stop=False, skip_group_check=True)
                        # L2: 20-row contraction against the next time tile;
                        # out tile 15 has no L2 term (zero tail padding).
                        nc.tensor.matmul(
                            pt[S - L2W:S, 0:(NI - 1) * B],
                            lhsT=bbr[0:CTX, fi, S:PW],
                            rhs=xqr[0:CTX, 1:NI, :, fi],
                            start=False, stop=True, skip_group_check=True)
                        if fi % 2 == 0:
                            nc.vector.tensor_copy(ysr[:, :, :, fi % fq2],
                                                  pt[:, :])
                        else:
                            nc.scalar.copy(ysr[:, :, :, fi % fq2], pt[:, :])
                        if fi % fq2 == fq2 - 1:
                            h = fi // fq2
                            dst = y_d.ap()[yo[ci] + h * S * nib * fq2:
                                           yo[ci] + (h + 1) * S * nib * fq2] \
                                .rearrange("(s m) -> s m", s=S, m=nib * fq2)
                            nc.sync.dma_start(out=dst, in_=ysb[:])

    nc.compile()
    _MODULE_CACHE[key] = nc
    return nc


def prep_x(x):
    """x (2048, 16, 1024) -> per-core flat arrays (chunk-major)."""
    xr = np.asarray(x, dtype=np.float32).reshape(NI, S, B, NCORES, FC)
    out = []
    for c in range(NCORES):
        parts = []
        f0 = 0
        for fq in CHUNKS:
            blk = xr[:, :, :, c, f0:f0 + fq]      # (i, s, b, f)
            parts.append(np.ascontiguousarray(blk).ravel())
            f0 += fq
        out.append(np.concatenate(parts))
    return np.stack(out)


def prep_bands(weight):
    """weight (1024, 21) -> per-core flat banded matrices (chunk-major)."""
    w = np.asarray(weight, dtype=np.float32).reshape(NCORES, FC, K)
    band = np.zeros((NCORES, FC, SB, S), np.float32)
    tau = np.arange(S)
    for k in range(K):
        band[:, :, tau + k, tau] = w[:, :, k][..., None]
    return band.reshape(NCORES, FC * SB * S)


def assemble_y(shards):
    """per-core flat y -> (2048, 16, 1024)."""
    y = np.empty((NI, S, B, NCORES, FC), np.float32)  # (i, tau, b, c, f)
    for c in range(NCORES):
        flat = np.asarray(shards[c]).ravel()
        f0 = 0
        o = 0
        for fq in CHUNKS:
            fq2 = fq // YS
            for h in range(YS):
                n = S * NI * B * fq2
                blk = flat[o:o + n].reshape(S, NI, B, fq2)  # (tau, i, b, f)
                y[:, :, :, c, f0:f0 + fq2] = blk.transpose(1, 0, 2, 3)
                o += n
                f0 += fq2
    return np.ascontiguousarray(y.reshape(T, B, F))


def kernel(x, weight, tail_padding):
    from concourse.bass_utils import run_bass_kernel_spmd

    nc = build_module()
    xs = prep_x(x)
    bs = prep_bands(weight)
    in_maps = [{"x": xs[c], "bands": bs[c]} for c in range(NCORES)]
    res = run_bass_kernel_spmd(nc, in_maps, list(range(NCORES)))
    shards = [res.results[c]["y"] for c in range(NCORES)]
    y = assemble_y(shards)
    seq_len = T if int(np.asarray(tail_padding)) else T - CTX
    return y[:seq_len]
